# revision 14
# baseline (speedup 1.0000x reference)
"""Trainium2 Bass kernel for the ContrastiveLoss problem.

Reference semantics (N=M=8192, D=512, C=1000):
    valid = labels1 > 0 ; n = sum(valid)
    sim   = inputs1 @ inputs2.T                       # [N, M]
    same  = labels1[:, None] == labels2[None, :]
    pos_sel = same  & (sim < 1 - EPS - POS_MARGIN) & valid[:, None]
    neg_sel = ~same & (sim > MARGIN)               & valid[:, None]
    loss = (sum(1-sim | pos_sel) + sum(sim | neg_sel)) / n
    avg_neg = count(neg_sel) / n
    avg_pos = round(100 * count(pos_sel) / n) / 100

Strategy (8 NeuronCores, data-parallel over rows of inputs1):
  * Host masks invalid rows into the operands (x1 row := 0, label := -1),
    so the device needs no validity logic at all.
  * Each core computes its [1024, 8192] slice of sim with bf16 matmuls
    (fp32 PSUM accumulation).
  * ScalarE copies PSUM -> SBUF as bf16; that copy is DMA'd to DRAM
    ("sdump") and is also the input for the reductions.
  * VectorE does two fused elementwise+row-reduce passes per tile:
      sum(relu(s - MARGIN)) and count(s > MARGIN)
    which give sum(s * [s > MARGIN]) = sum(relu(s-MARGIN)) + MARGIN*count
    over ALL entries (ignoring `same`).
  * `same` entries are ~1/1000 of the matrix and depend only on the
    labels, which the host knows. The host gathers those ~67k sim values
    from sdump and applies the exact pos/neg corrections in numpy.
"""

import numpy as np
import ml_dtypes

N, M, D = 8192, 8192, 512
NCORES = 8
ROWS = N // NCORES  # rows of inputs1 per core
MARGIN = 0.5
POS_MARGIN = 0.05
EPS = 1e-6

DCH = D // 128     # contraction chunks (partition dim is 128)
MT = ROWS // 128   # row tiles per core
JG = 4             # column groups (each spans 4 PSUM banks)
JW = M // JG       # columns per group
NMM = JW // 512    # matmuls (N=512) per group
NSLOT = MT * JG * 2

_NC = None


def _build_program():
    import concourse.tile as tile
    from concourse import bacc, mybir

    nc = bacc.Bacc(
        "TRN2", target_bir_lowering=False, debug=False, num_devices=NCORES
    )
    bf16 = mybir.dt.bfloat16
    f32 = mybir.dt.float32

    # const AP for the Sign activation's bias
    _bias = nc.alloc_sbuf_tensor("const-float32-negmargin", [128, 1], f32)
    nc.gpsimd.memset(_bias.ap(), -float(MARGIN))
    nc.const_aps.aps[(f32, -float(MARGIN))] = _bias.ap()
    nc.all_engine_barrier()

    x1t = nc.dram_tensor("x1t", [D, ROWS], bf16, kind="ExternalInput").ap()
    x2t = nc.dram_tensor("x2t", [D, M], bf16, kind="ExternalInput").ap()
    sdump = nc.dram_tensor("sdump", [ROWS, M], bf16, kind="ExternalOutput").ap()
    NACC = MT * JG
    stats_r = nc.dram_tensor("stats_r", [128, NACC], f32, kind="ExternalOutput").ap()
    stats_c = nc.dram_tensor("stats_c", [128, NACC], f32, kind="ExternalOutput").ap()
    stats_s = nc.dram_tensor("stats_s", [128, NACC], f32, kind="ExternalOutput").ap()

    with tile.TileContext(nc) as tc:
        with (
            tc.tile_pool(name="x1p", bufs=1) as x1p,
            tc.tile_pool(name="x2p", bufs=1) as x2p,
            tc.tile_pool(name="psp", bufs=2, space="PSUM") as psp,
            tc.tile_pool(name="sbp", bufs=4) as sbp,
            tc.tile_pool(name="scp", bufs=2) as scp,
            tc.tile_pool(name="scp2", bufs=2) as scp2,
            tc.tile_pool(name="stp", bufs=1) as stp,
        ):
            # weights: [k=128, d, m] ; rhs: [k=128, d, j]
            x1s = x1p.tile([128, DCH, ROWS], bf16)
            nc.sync.dma_start(x1s[:], x1t.rearrange("(d p) m -> p d m", p=128))
            x2s = x2p.tile([128, DCH, M], bf16)
            x2v = x2t.rearrange("(d p) j -> p d j", p=128)
            for jc in range(JG):
                nc.sync.dma_start(
                    x2s[:, :, jc * JW : (jc + 1) * JW],
                    x2v[:, :, jc * JW : (jc + 1) * JW],
                )

            stats_rt = stp.tile([128, NACC], f32, tag="str")
            stats_ct = stp.tile([128, NACC], f32, tag="stc")
            stats_st = stp.tile([128, NACC], f32, tag="sts")

            for m in range(MT):
                for jg in range(JG):
                    ps = psp.tile([128, JW], f32)
                    for d in range(DCH):
                        for jj in range(NMM):
                            nc.tensor.matmul(
                                ps[:, jj * 512 : (jj + 1) * 512],
                                x1s[:, d, m * 128 : (m + 1) * 128],
                                x2s[:, d, jg * JW + jj * 512 : jg * JW + (jj + 1) * 512],
                                start=(d == 0),
                                stop=(d == DCH - 1),
                            )
                    sb = sbp.tile([128, JW], bf16)
                    nc.scalar.copy(sb[:], ps[:])
                    nc.sync.dma_start(
                        sdump[m * 128 : (m + 1) * 128, jg * JW : (jg + 1) * JW],
                        sb[:],
                    )
                    slot = m * JG + jg
                    # All reduce passes read PSUM directly (the reduce uop is
                    # 1x regardless of source) so they don't serialize behind
                    # the ScalarE copy.
                    scr = scp.tile([128, JW], bf16, tag="scr")
                    nc.vector.tensor_scalar(
                        scr[:],
                        ps[:],
                        float(MARGIN),
                        0.0,
                        mybir.AluOpType.subtract,
                        mybir.AluOpType.max,
                        accum_out=stats_rt[:, slot : slot + 1],
                    )
                    scr2 = scp2.tile([128, JW], bf16, tag="scr2")
                    # Bresenham split of the 32 count passes: ~19 on ScalarE,
                    # 13 on VectorE, to balance engine busy time.
                    on_act = ((slot + 1) * 19) // 32 > (slot * 19) // 32
                    if on_act:
                        # count via ScalarE: sum(sign(s - MARGIN)) = 2*count - K
                        nc.scalar.activation(
                            scr2[:],
                            ps[:],
                            mybir.ActivationFunctionType.Sign,
                            bias=-float(MARGIN),
                            accum_out=stats_st[:, slot : slot + 1],
                        )
                    else:
                        nc.vector.tensor_scalar(
                            scr2[:],
                            ps[:],
                            float(MARGIN),
                            0.0,
                            mybir.AluOpType.is_gt,
                            mybir.AluOpType.max,
                            accum_out=stats_ct[:, slot : slot + 1],
                        )

            nc.sync.dma_start(stats_r[:], stats_rt[:])
            nc.sync.dma_start(stats_c[:], stats_ct[:])
            nc.sync.dma_start(stats_s[:], stats_st[:])

    nc.compile()
    return nc


def _get_program():
    global _NC
    if _NC is None:
        _NC = _build_program()
    return _NC


def run(inputs, trace=False):
    from concourse.bass_utils import run_bass_kernel_spmd

    x1 = np.asarray(inputs["inputs1"], dtype=np.float32)
    l1 = np.asarray(inputs["labels1"]).astype(np.int64)
    x2 = np.asarray(inputs["inputs2"], dtype=np.float32)
    l2 = np.asarray(inputs["labels2"]).astype(np.int64)

    valid = l1 > 0
    n = int(valid.sum())

    # Fold the row-validity mask into the operands: sim rows of invalid
    # rows become 0 (-> no neg contribution) and their label -1 never
    # matches labels2 (-> no pos contribution).
    x1m = np.where(valid[:, None], x1, np.float32(0)).astype(ml_dtypes.bfloat16)
    x2b = x2.astype(ml_dtypes.bfloat16)
    x1T = np.ascontiguousarray(x1m.T)  # [D, N]
    x2T = np.ascontiguousarray(x2b.T)  # [D, M]

    nc = _get_program()
    in_maps = [
        {
            "x1t": np.ascontiguousarray(x1T[:, c * ROWS : (c + 1) * ROWS]),
            "x2t": x2T,
        }
        for c in range(NCORES)
    ]
    res = run_bass_kernel_spmd(nc, in_maps, core_ids=list(range(NCORES)), trace=trace)

    # --- combine the dense (same-agnostic) partial sums ---
    nacc = MT * JG
    slots = np.arange(nacc)
    on_act = ((slots + 1) * 19) // 32 > (slots * 19) // 32
    relu_sum = 0.0
    cnt_sum = 0.0
    for c in range(NCORES):
        relu_sum += res.results[c]["stats_r"].astype(np.float64).sum()
        cnt_sum += res.results[c]["stats_c"].astype(np.float64)[:, ~on_act].sum()
        sign_sum = res.results[c]["stats_s"].astype(np.float64)[:, on_act].sum()
        # sum(sign(s - MARGIN)) over the ScalarE slots = 2*count - n_elements
        cnt_sum += (sign_sum + int(on_act.sum()) * 128 * JW) / 2.0
    neg_val = relu_sum + MARGIN * cnt_sum  # sum(s * [s > MARGIN]) over all pairs
    neg_cnt = cnt_sum

    # --- sparse same-label corrections from the dumped sim values ---
    l1m = np.where(valid, l1, -1)
    sort_idx = np.argsort(l2, kind="stable")
    sl2 = l2[sort_idx]
    lo = np.searchsorted(sl2, l1m, "left")
    hi = np.searchsorted(sl2, l1m, "right")
    pos_thresh = np.float32(1.0) - np.float32(EPS) - np.float32(POS_MARGIN)

    pos_loss = 0.0
    pos_cnt = 0
    for c in range(NCORES):
        r0 = c * ROWS
        clo, chi = lo[r0 : r0 + ROWS], hi[r0 : r0 + ROWS]
        cnts = chi - clo
        if cnts.sum() == 0:
            continue
        col_list = np.concatenate(
            [sort_idx[a:b] for a, b in zip(clo, chi) if b > a]
        )
        row_list = np.repeat(np.arange(ROWS), cnts)
        sd = res.results[c]["sdump"]
        s = sd[row_list, col_list].astype(np.float64)
        pm = s < pos_thresh
        pos_loss += (1.0 - s[pm]).sum()
        pos_cnt += int(pm.sum())
        # remove the same-label entries the dense pass wrongly counted as neg
        nm = s > MARGIN
        neg_val -= s[nm].sum()
        neg_cnt -= int(nm.sum())

    loss = np.float32((pos_loss + neg_val) / n)
    avg_neg = np.float32(neg_cnt / n)
    avg_pos = np.float32(np.round(100.0 * pos_cnt / n) / 100.0)
    out = (
        np.array(loss, dtype=np.float32),
        np.array(avg_neg, dtype=np.float32),
        np.array(avg_pos, dtype=np.float32),
    )
    return out, res


def kernel(**inputs):
    out, _ = run(inputs)
    return out


# revision 15
# speedup vs baseline: 1.6338x; 1.6338x over previous
"""Trainium2 Bass kernel for the ContrastiveLoss problem.

Reference semantics (N=M=8192, D=512, C=1000):
    valid = labels1 > 0 ; n = sum(valid)
    sim   = inputs1 @ inputs2.T                       # [N, M]
    same  = labels1[:, None] == labels2[None, :]
    pos_sel = same  & (sim < 1 - EPS - POS_MARGIN) & valid[:, None]
    neg_sel = ~same & (sim > MARGIN)               & valid[:, None]
    loss = (sum(1-sim | pos_sel) + sum(sim | neg_sel)) / n
    avg_neg = count(neg_sel) / n
    avg_pos = round(100 * count(pos_sel) / n) / 100

Strategy (8 NeuronCores, data-parallel over rows of inputs1):
  * Host masks invalid rows into the operands (x1 row := 0, label := -1),
    so the device needs no validity logic at all.
  * Each core computes its [1024, 8192] slice of sim with bf16 matmuls
    (fp32 PSUM accumulation).
  * ScalarE copies PSUM -> SBUF as bf16; that copy is DMA'd to DRAM
    ("sdump") and is also the input for the reductions.
  * VectorE does two fused elementwise+row-reduce passes per tile:
      sum(relu(s - MARGIN)) and count(s > MARGIN)
    which give sum(s * [s > MARGIN]) = sum(relu(s-MARGIN)) + MARGIN*count
    over ALL entries (ignoring `same`).
  * `same` entries are ~1/1000 of the matrix and depend only on the
    labels, which the host knows. The host gathers those ~67k sim values
    from sdump and applies the exact pos/neg corrections in numpy.
"""

import numpy as np
import ml_dtypes

N, M, D = 8192, 8192, 512
NCORES = 8
ROWS = N // NCORES  # rows of inputs1 per core
MARGIN = 0.5
POS_MARGIN = 0.05
EPS = 1e-6

DCH = D // 128     # contraction chunks (partition dim is 128)
MT = ROWS // 128   # row tiles per core
JG = 4             # column groups (each spans 4 PSUM banks)
JW = M // JG       # columns per group
NMM = JW // 512    # matmuls (N=512) per group
NSLOT = MT * JG * 2

_NC = None


def _build_program():
    import concourse.tile as tile
    from concourse import bacc, mybir

    nc = bacc.Bacc(
        "TRN2", target_bir_lowering=False, debug=False, num_devices=NCORES
    )
    bf16 = mybir.dt.bfloat16
    f32 = mybir.dt.float32

    # const AP for the Sign activation's bias
    _bias = nc.alloc_sbuf_tensor("const-float32-negmargin", [128, 1], f32)
    nc.gpsimd.memset(_bias.ap(), -float(MARGIN))
    nc.const_aps.aps[(f32, -float(MARGIN))] = _bias.ap()
    nc.all_engine_barrier()

    x1t = nc.dram_tensor("x1t", [D, ROWS], bf16, kind="ExternalInput").ap()
    x2t = nc.dram_tensor("x2t", [D, M], bf16, kind="ExternalInput").ap()
    sdump = nc.dram_tensor("sdump", [ROWS, M], bf16, kind="ExternalOutput").ap()
    NACC = MT * JG
    stats_r = nc.dram_tensor("stats_r", [128, NACC], f32, kind="ExternalOutput").ap()
    stats_c = nc.dram_tensor("stats_c", [128, NACC], f32, kind="ExternalOutput").ap()
    stats_s = nc.dram_tensor("stats_s", [128, NACC], f32, kind="ExternalOutput").ap()

    with tile.TileContext(nc) as tc:
        with (
            tc.tile_pool(name="x1p", bufs=1) as x1p,
            tc.tile_pool(name="x2p", bufs=1) as x2p,
            tc.tile_pool(name="psp", bufs=2, space="PSUM") as psp,
            tc.tile_pool(name="sbp", bufs=4) as sbp,
            tc.tile_pool(name="scp", bufs=2) as scp,
            tc.tile_pool(name="scp2", bufs=2) as scp2,
            tc.tile_pool(name="stp", bufs=1) as stp,
        ):
            # weights: [k=128, d, m] ; rhs: [k=128, d, j]
            x1s = x1p.tile([128, DCH, ROWS], bf16)
            nc.sync.dma_start(x1s[:], x1t.rearrange("(d p) m -> p d m", p=128))
            x2s = x2p.tile([128, DCH, M], bf16)
            x2v = x2t.rearrange("(d p) j -> p d j", p=128)
            for jc in range(JG):
                nc.sync.dma_start(
                    x2s[:, :, jc * JW : (jc + 1) * JW],
                    x2v[:, :, jc * JW : (jc + 1) * JW],
                )

            stats_rt = stp.tile([128, NACC], f32, tag="str")
            stats_ct = stp.tile([128, NACC], f32, tag="stc")
            stats_st = stp.tile([128, NACC], f32, tag="sts")

            for m in range(MT):
                for jg in range(JG):
                    ps = psp.tile([128, JW], f32)
                    for d in range(DCH):
                        for jj in range(NMM):
                            nc.tensor.matmul(
                                ps[:, jj * 512 : (jj + 1) * 512],
                                x1s[:, d, m * 128 : (m + 1) * 128],
                                x2s[:, d, jg * JW + jj * 512 : jg * JW + (jj + 1) * 512],
                                start=(d == 0),
                                stop=(d == DCH - 1),
                            )
                    sb = sbp.tile([128, JW], bf16)
                    nc.scalar.copy(sb[:], ps[:])
                    nc.sync.dma_start(
                        sdump[m * 128 : (m + 1) * 128, jg * JW : (jg + 1) * JW],
                        sb[:],
                    )
                    slot = m * JG + jg
                    scr = scp.tile([128, JW], bf16, tag="scr")
                    nc.vector.tensor_scalar(
                        scr[:],
                        sb[:],
                        float(MARGIN),
                        0.0,
                        mybir.AluOpType.subtract,
                        mybir.AluOpType.max,
                        accum_out=stats_rt[:, slot : slot + 1],
                    )
                    scr2 = scp2.tile([128, JW], bf16, tag="scr2")
                    # Bresenham split of the 32 count passes: ~19 on ScalarE,
                    # 13 on VectorE, to balance engine busy time.
                    on_act = ((slot + 1) * 19) // 32 > (slot * 19) // 32
                    if on_act:
                        # count via ScalarE: sum(sign(s - MARGIN)) = 2*count - K
                        nc.scalar.activation(
                            scr2[:],
                            sb[:],
                            mybir.ActivationFunctionType.Sign,
                            bias=-float(MARGIN),
                            accum_out=stats_st[:, slot : slot + 1],
                        )
                    else:
                        nc.vector.tensor_scalar(
                            scr2[:],
                            sb[:],
                            float(MARGIN),
                            0.0,
                            mybir.AluOpType.is_gt,
                            mybir.AluOpType.max,
                            accum_out=stats_ct[:, slot : slot + 1],
                        )

            nc.sync.dma_start(stats_r[:], stats_rt[:])
            nc.sync.dma_start(stats_c[:], stats_ct[:])
            nc.sync.dma_start(stats_s[:], stats_st[:])

    nc.compile()
    return nc


def _get_program():
    global _NC
    if _NC is None:
        _NC = _build_program()
    return _NC


def run(inputs, trace=False):
    from concourse.bass_utils import run_bass_kernel_spmd

    x1 = np.asarray(inputs["inputs1"], dtype=np.float32)
    l1 = np.asarray(inputs["labels1"]).astype(np.int64)
    x2 = np.asarray(inputs["inputs2"], dtype=np.float32)
    l2 = np.asarray(inputs["labels2"]).astype(np.int64)

    valid = l1 > 0
    n = int(valid.sum())

    # Fold the row-validity mask into the operands: sim rows of invalid
    # rows become 0 (-> no neg contribution) and their label -1 never
    # matches labels2 (-> no pos contribution).
    x1m = np.where(valid[:, None], x1, np.float32(0)).astype(ml_dtypes.bfloat16)
    x2b = x2.astype(ml_dtypes.bfloat16)
    x1T = np.ascontiguousarray(x1m.T)  # [D, N]
    x2T = np.ascontiguousarray(x2b.T)  # [D, M]

    nc = _get_program()
    in_maps = [
        {
            "x1t": np.ascontiguousarray(x1T[:, c * ROWS : (c + 1) * ROWS]),
            "x2t": x2T,
        }
        for c in range(NCORES)
    ]
    res = run_bass_kernel_spmd(nc, in_maps, core_ids=list(range(NCORES)), trace=trace)

    # --- combine the dense (same-agnostic) partial sums ---
    nacc = MT * JG
    slots = np.arange(nacc)
    on_act = ((slots + 1) * 19) // 32 > (slots * 19) // 32
    relu_sum = 0.0
    cnt_sum = 0.0
    for c in range(NCORES):
        relu_sum += res.results[c]["stats_r"].astype(np.float64).sum()
        cnt_sum += res.results[c]["stats_c"].astype(np.float64)[:, ~on_act].sum()
        sign_sum = res.results[c]["stats_s"].astype(np.float64)[:, on_act].sum()
        # sum(sign(s - MARGIN)) over the ScalarE slots = 2*count - n_elements
        cnt_sum += (sign_sum + int(on_act.sum()) * 128 * JW) / 2.0
    neg_val = relu_sum + MARGIN * cnt_sum  # sum(s * [s > MARGIN]) over all pairs
    neg_cnt = cnt_sum

    # --- sparse same-label corrections from the dumped sim values ---
    l1m = np.where(valid, l1, -1)
    sort_idx = np.argsort(l2, kind="stable")
    sl2 = l2[sort_idx]
    lo = np.searchsorted(sl2, l1m, "left")
    hi = np.searchsorted(sl2, l1m, "right")
    pos_thresh = np.float32(1.0) - np.float32(EPS) - np.float32(POS_MARGIN)

    pos_loss = 0.0
    pos_cnt = 0
    for c in range(NCORES):
        r0 = c * ROWS
        clo, chi = lo[r0 : r0 + ROWS], hi[r0 : r0 + ROWS]
        cnts = chi - clo
        if cnts.sum() == 0:
            continue
        col_list = np.concatenate(
            [sort_idx[a:b] for a, b in zip(clo, chi) if b > a]
        )
        row_list = np.repeat(np.arange(ROWS), cnts)
        sd = res.results[c]["sdump"]
        s = sd[row_list, col_list].astype(np.float64)
        pm = s < pos_thresh
        pos_loss += (1.0 - s[pm]).sum()
        pos_cnt += int(pm.sum())
        # remove the same-label entries the dense pass wrongly counted as neg
        nm = s > MARGIN
        neg_val -= s[nm].sum()
        neg_cnt -= int(nm.sum())

    loss = np.float32((pos_loss + neg_val) / n)
    avg_neg = np.float32(neg_cnt / n)
    avg_pos = np.float32(np.round(100.0 * pos_cnt / n) / 100.0)
    out = (
        np.array(loss, dtype=np.float32),
        np.array(avg_neg, dtype=np.float32),
        np.array(avg_pos, dtype=np.float32),
    )
    return out, res


def kernel(**inputs):
    out, _ = run(inputs)
    return out
